# revision 3
# baseline (speedup 1.0000x reference)
"""Trainium2 Bass kernel for nn_ColumnUniform (GNN message passing), v2.

out[e] = edge_attr[e] / rowsum(edge_attr)[col[e]]   for 20M edges, 1M nodes.

Sharding: node range per core (8 cores). Core c gets the edges whose ROW is
in its range (A stream, for rowsums) and the edges whose COL is in its range
(B stream, for scaling); the reciprocal table is produced and consumed on
the same core, so there is no inter-core communication.

Layout (host, pure index manipulation; fp16 wire format):
  Nodes are classed by row degree (<=NCD DP-chosen ceilings D) and by col
  degree (<=NCE ceilings E); cell = (dclass, eclass), D-primary order.
  Per cell: K = ceil(max-core count/128) slot columns; node k -> partition
  k%128, column k//128. The v table [128, WV] has one slot per node.
    A stream: slot-major windows, edge i of row r at aoff + j*D + i.
      One windowed tensor_reduce per D-class (its cells are contiguous).
    B stream: plane-interleaved, edge i of col c at boff + i*K + j.
      One broadcast multiply per cell: [P, E, K] *= vh[:, v0:v0+K].
  Reciprocal + f32->f16 convert run on the scalar (Act) engine, off the
  DVE critical path. Stores stream out region-by-region behind the muls.
"""
import sys

for _p in ("/opt/trn_rl_repo", "/root/.axon_site/_ro/trn_rl_repo"):
    if _p not in sys.path:
        sys.path.append(_p)

import numpy as np

import concourse.bass as bass
import concourse.mybir as mybir
from concourse.bass_utils import run_bass_kernel_spmd

F32 = mybir.dt.float32
F16 = mybir.dt.float16

P = 128
N_CORES = 8
NCD = 8                # row-degree classes (A side)
NCE = 4                # col-degree classes (B side)
CHUNK = 4096           # load chunk width (f16 columns)
NREGION = 10           # output store regions


# ----------------------------------------------------------------------------
# Host-side layout: integer index work only.
# ----------------------------------------------------------------------------

def dp_classes(deg, K):
    deg = deg[deg > 0]
    dmax = int(deg.max())
    cnt = np.bincount(deg, minlength=dmax + 1).astype(np.int64)
    vals = np.nonzero(cnt)[0]
    vals = vals[vals > 0]
    csum = np.concatenate([[0], np.cumsum(cnt)])
    M = len(vals)
    INF = float("inf")
    dp = np.full((K + 1, M), INF)
    par = np.zeros((K + 1, M), np.int64)
    for j in range(M):
        dp[1][j] = csum[vals[j] + 1] * vals[j]
    for k in range(2, K + 1):
        for j in range(k - 1, M):
            costs = dp[k - 1][:j] + (csum[vals[j] + 1] - csum[vals[:j] + 1]) * vals[j]
            i = int(np.argmin(costs))
            dp[k][j] = costs[i]
            par[k][j] = i
    k = int(np.argmin(dp[:, M - 1]))
    out = []
    j = M - 1
    while k >= 1:
        out.append(int(vals[j]))
        j = int(par[k][j])
        k -= 1
    return np.array(sorted(out), np.int64)


def edge_ranks(keys, N, E):
    ptr = np.zeros(N + 1, np.int64)
    np.cumsum(np.bincount(keys, minlength=N), out=ptr[1:])
    prm = np.argsort(keys, kind="stable")
    r = np.arange(E, dtype=np.int64) - ptr[keys[prm]]
    out = np.empty(E, np.int64)
    out[prm] = r
    return out


def prepare(edge_index, edge_attr, n_nodes):
    row = np.asarray(edge_index[0]).astype(np.int64)
    col = np.asarray(edge_index[1]).astype(np.int64)
    attr16 = np.asarray(edge_attr, dtype=np.float32).astype(np.float16)
    E = row.shape[0]
    N = int(n_nodes)
    NR = (N + N_CORES - 1) // N_CORES

    rd = np.bincount(row, minlength=N)
    cd = np.bincount(col, minlength=N)
    clD = dp_classes(rd, NCD)
    clE = dp_classes(cd, NCE)
    ncd, nce = len(clD), len(clE)
    dcls = np.searchsorted(clD, np.maximum(rd, 1))
    ecls = np.searchsorted(clE, np.maximum(cd, 1))
    cell = dcls * nce + ecls
    NCELL = ncd * nce
    core = np.minimum(np.arange(N) // NR, N_CORES - 1)

    counts = np.zeros((N_CORES, NCELL), np.int64)
    np.add.at(counts, (core, cell), 1)
    order = np.lexsort((np.arange(N), cell, core))
    grp = core[order] * NCELL + cell[order]
    starts = np.concatenate([[0], np.nonzero(np.diff(grp))[0] + 1])
    gstart = np.zeros(N, np.int64)
    gstart[starts] = starts
    np.maximum.accumulate(gstart, out=gstart)
    kn = np.empty(N, np.int64)
    kn[order] = np.arange(N) - gstart

    K = -(-counts.max(axis=0) // P)
    Dc = clD[np.arange(NCELL) // nce]
    Ec = clE[np.arange(NCELL) % nce]
    cv = np.concatenate([[0], np.cumsum(K)])
    aoff = np.concatenate([[0], np.cumsum(K * Dc)])
    boff = np.concatenate([[0], np.cumsum(K * Ec)])
    WV, WA, WB = int(cv[-1]), int(aoff[-1]), int(boff[-1])

    pn = kn % P
    jn = kn // P

    rrank = edge_ranks(row, N, E)
    crank = edge_ranks(col, N, E)

    acol = aoff[cell[row]] + jn[row] * Dc[cell[row]] + rrank
    fa = core[row] * (P * WA) + pn[row] * WA + acol
    attr_a = np.zeros(N_CORES * P * WA, np.float16)
    attr_a[fa] = attr16
    attr_a = attr_a.reshape(N_CORES, P, WA)

    bcol = boff[cell[col]] + crank * K[cell[col]] + jn[col]
    fb = core[col] * (P * WB) + pn[col] * WB + bcol
    attr_b = np.zeros(N_CORES * P * WB, np.float16)
    attr_b[fb] = attr16
    attr_b = attr_b.reshape(N_CORES, P, WB)

    classes = []
    for d in range(ncd):
        c0, c1 = d * nce, (d + 1) * nce
        classes.append(dict(D=int(clD[d]), a0=int(aoff[c0]), a1=int(aoff[c1]),
                            v0=int(cv[c0]), v1=int(cv[c1])))
    cells = []
    for c in range(NCELL):
        if K[c] == 0 or Ec[c] == 0:
            continue
        cells.append(dict(E=int(Ec[c]), K=int(K[c]), b0=int(boff[c]),
                          v0=int(cv[c]), d=int(c // nce)))
    geom = dict(WA=WA, WB=WB, WV=WV, classes=classes, cells=cells)
    binfo = (core[col], pn[col] * WB + bcol)
    in_maps = [{"attr_a": attr_a[c], "attr_b": attr_b[c]}
               for c in range(N_CORES)]
    return in_maps, geom, binfo


def unshard(results, geom, binfo):
    bcore, fb_local = binfo
    outs = np.stack([np.asarray(results[c]["out"]).reshape(-1)
                     for c in range(N_CORES)])
    return outs[bcore, fb_local].astype(np.float32)


# ----------------------------------------------------------------------------
# Device program
# ----------------------------------------------------------------------------

def build_program(geom):
    WA, WB, WV = geom["WA"], geom["WB"], geom["WV"]
    classes = geom["classes"]
    cells = geom["cells"]

    nc = bass.Bass()
    attr_a = nc.declare_dram_parameter("attr_a", [P, WA], F16, isOutput=False)
    attr_b = nc.declare_dram_parameter("attr_b", [P, WB], F16, isOutput=False)
    out_ext = nc.declare_dram_parameter("out", [P, WB], F16, isOutput=True)

    achunks = [(w, min(WA, w + CHUNK)) for w in range(0, WA, CHUNK)]
    bchunks = [(w, min(WB, w + CHUNK)) for w in range(0, WB, CHUNK)]
    na = len(achunks)

    # A chunk index needed by each class; B chunk index needed by each cell
    def a_hi(cl):
        return (cl["a1"] - 1) // CHUNK

    def b_hi(ce):
        return (ce["b0"] + ce["E"] * ce["K"] - 1) // CHUNK

    # output store regions at cell boundaries; finer near the end so the
    # final store (which trails the last multiply) is short.
    regions = []
    tgt = (WB + NREGION - 1) // NREGION
    r0, nmul = 0, 0
    for ci, ce in enumerate(cells):
        end = ce["b0"] + ce["E"] * ce["K"]
        nmul += 1
        left = WB - end
        cur_tgt = tgt if left > 2 * tgt else max(tgt // 4, 1)
        if end - r0 >= cur_tgt or ci == len(cells) - 1:
            regions.append((r0, end, nmul))
            r0 = end
    assert regions[-1][1] == WB

    from contextlib import ExitStack
    with ExitStack() as ctx:
        block = ctx.enter_context(nc.Block())
        sLoad = ctx.enter_context(nc.semaphore("sLoad"))
        sRed = ctx.enter_context(nc.semaphore("sRed"))
        sV = ctx.enter_context(nc.semaphore("sV"))
        sMul = ctx.enter_context(nc.semaphore("sMul"))
        sOut = ctx.enter_context(nc.semaphore("sOut"))

        A_sb = ctx.enter_context(nc.sbuf_tensor("A_sb", [P, WA], F16))
        B_sb = ctx.enter_context(nc.sbuf_tensor("B_sb", [P, WB], F16))
        rs = ctx.enter_context(nc.sbuf_tensor("rs", [P, WV], F32))
        v32 = ctx.enter_context(nc.sbuf_tensor("v32", [P, WV], F32))
        vh = ctx.enter_context(nc.sbuf_tensor("vh", [P, WV], F16))

        @block.sync
        def _(sync):
            for w0, w1 in achunks:
                sync.dma_start(out=A_sb[:, w0:w1],
                               in_=attr_a[:, w0:w1]).then_inc(sLoad, 16)
            for w0, w1 in bchunks:
                sync.dma_start(out=B_sb[:, w0:w1],
                               in_=attr_b[:, w0:w1]).then_inc(sLoad, 16)

        @block.vector
        def _(vector):
            waited = {}

            def wait(eng, sem, val):
                if waited.get(id(sem), -1) < val:
                    eng.wait_ge(sem, val)
                    waited[id(sem)] = val

            for cl in classes:
                wait(vector, sLoad, 16 * (a_hi(cl) + 1))
                D, a0, a1, v0, v1 = cl["D"], cl["a0"], cl["a1"], cl["v0"], cl["v1"]
                src = A_sb[:, a0:a1].rearrange("p (k d) -> p k d", d=D)
                vector.tensor_reduce(
                    out=rs[:, v0:v1], in_=src,
                    axis=mybir.AxisListType.X, op=mybir.AluOpType.add,
                ).then_inc(sRed, 1)
            for ce in cells:
                wait(vector, sLoad, 16 * (na + b_hi(ce) + 1))
                wait(vector, sV, ce["d"] + 1)
                E, K, b0, v0 = ce["E"], ce["K"], ce["b0"], ce["v0"]
                dst = B_sb[:, b0:b0 + E * K].rearrange("p (e k) -> p e k", k=K)
                vector.tensor_tensor(
                    out=dst, in0=vh[:, None, v0:v0 + K].to_broadcast([P, E, K]),
                    in1=dst, op=mybir.AluOpType.mult,
                ).then_inc(sMul, 1)

        @block.scalar
        def _(scalar):
            for d, cl in enumerate(classes):
                scalar.wait_ge(sRed, d + 1)
                v0, v1 = cl["v0"], cl["v1"]
                scalar.add_instruction(mybir.InstActivation(
                    name=nc.get_next_instruction_name(),
                    func=mybir.ActivationFunctionType.Reciprocal,
                    ins=[scalar.lower_ap(rs[:, v0:v1]),
                         mybir.ImmediateValue(dtype=F32, value=0.0),
                         mybir.ImmediateValue(dtype=F32, value=1.0),
                         mybir.ImmediateValue(dtype=F32, value=0.0)],
                    outs=[scalar.lower_ap(v32[:, v0:v1])]))
                scalar.activation(out=vh[:, v0:v1], in_=v32[:, v0:v1],
                                  func=mybir.ActivationFunctionType.Copy,
                                  ).then_inc(sV, 1)
            for r0, r1, nmul in regions:
                scalar.wait_ge(sMul, nmul)
                scalar.dma_start(out=out_ext[:, r0:r1],
                                 in_=B_sb[:, r0:r1]).then_inc(sOut, 16)

    return nc


# ----------------------------------------------------------------------------
# Entry point
# ----------------------------------------------------------------------------

def kernel(edge_index, edge_attr, N):
    import os
    in_maps, geom, binfo = prepare(edge_index, edge_attr, int(N))
    nc = build_program(geom)
    trace = os.environ.get("KTRACE") not in (None, "", "0")
    if trace:
        import types
        import antenv
        if "antenv.axon_hooks" not in sys.modules:
            mod = types.ModuleType("antenv.axon_hooks")
            _h = [None]
            mod.set_axon_ntff_profile_hook = lambda h: _h.__setitem__(0, h)
            mod.get_axon_ntff_profile_hook = lambda: _h[0]
            sys.modules["antenv.axon_hooks"] = mod
            antenv.axon_hooks = mod
            from trn_agent_boot.trn_boot import _ntff_profile_via_ctypes
            mod.set_axon_ntff_profile_hook(
                _ntff_profile_via_ctypes("/opt/axon/libaxon_pjrt.so"))
    res = run_bass_kernel_spmd(nc, in_maps, list(range(N_CORES)), trace=trace)
    kernel.last = (res, in_maps, geom)
    return unshard(res.results, geom, binfo)


if __name__ == "__main__":
    rng = np.random.default_rng(0)
    N = 4096
    E = 65536
    row = np.concatenate([np.arange(N, dtype=np.int32),
                          rng.integers(0, N, E - N, dtype=np.int32)])
    col = rng.integers(0, N, E, dtype=np.int32)
    attr = rng.random(E, dtype=np.float32) * 0.9 + 0.1
    out = kernel(np.stack([row, col]), attr, N)
    rowsum = np.zeros(N, np.float64)
    np.add.at(rowsum, row, attr.astype(np.float64))
    exp = (1.0 / rowsum)[col] * attr
    err = np.abs(out - exp) / np.abs(exp)
    print("max rel err:", err.max())


# revision 4
# speedup vs baseline: 1.2332x; 1.2332x over previous
"""Trainium2 Bass kernel for nn_ColumnUniform (GNN message passing), v4.

out[e] = edge_attr[e] / rowsum(edge_attr)[col[e]]   for 20M edges, 1M nodes.

Sharding: nodes are dealt round-robin to the 8 cores within each (row-degree
class, col-degree class) cell, so every cell is balanced across cores. A core
receives the edges whose ROW node it owns (A stream, for rowsums) and the
edges whose COL node it owns (B stream, for scaling); the reciprocal table is
produced and consumed on the same core — no inter-core communication.

fp16 wire format throughout (tolerance 2e-2 >> f16 rounding).

Device pipeline (per core):
  - A stream [128, WA] f16, plane-interleaved per D-class: plane i holds the
    i-th row-edge of every slot. The TENSOR engine reduces: per class, D
    matmuls with an identity stationary accumulate the planes into PSUM
    (f32), giving rowsums at the v-table slots. DVE is not involved.
  - Scalar engine: per class, Activation-Reciprocal PSUM->vh f16 (measured
    ~1e-5 accurate before the f16 rounding).
  - B stream [128, WB] f16, plane-interleaved per (D,E) cell. DVE does one
    broadcast multiply per cell: [P, E, K] *= vh[:, v0:v0+K]  (f16 2x mode).
  - Stores stream out region-by-region behind the multiplies (scalar queue).
"""
import sys

for _p in ("/opt/trn_rl_repo", "/root/.axon_site/_ro/trn_rl_repo"):
    if _p not in sys.path:
        sys.path.append(_p)

import numpy as np

import concourse.bass as bass
import concourse.mybir as mybir
from concourse.bass_utils import run_bass_kernel_spmd

F32 = mybir.dt.float32
F16 = mybir.dt.float16

P = 128
N_CORES = 8
NCD = 6                # row-degree classes (A side)
NCE = 8                # col-degree classes (B side)
CHUNK = 4096           # load chunk width (f16 columns)
NREGION = 10           # output store regions
PSUM_COLS = 512        # f32 columns per PSUM bank


# ----------------------------------------------------------------------------
# Host-side layout: integer index work only.
# ----------------------------------------------------------------------------

def dp_classes(deg, K):
    deg = deg[deg > 0]
    dmax = int(deg.max())
    cnt = np.bincount(deg, minlength=dmax + 1).astype(np.int64)
    vals = np.nonzero(cnt)[0]
    vals = vals[vals > 0]
    csum = np.concatenate([[0], np.cumsum(cnt)])
    M = len(vals)
    INF = float("inf")
    dp = np.full((K + 1, M), INF)
    par = np.zeros((K + 1, M), np.int64)
    for j in range(M):
        dp[1][j] = csum[vals[j] + 1] * vals[j]
    for k in range(2, K + 1):
        for j in range(k - 1, M):
            costs = dp[k - 1][:j] + (csum[vals[j] + 1] - csum[vals[:j] + 1]) * vals[j]
            i = int(np.argmin(costs))
            dp[k][j] = costs[i]
            par[k][j] = i
    k = int(np.argmin(dp[:, M - 1]))
    out = []
    j = M - 1
    while k >= 1:
        out.append(int(vals[j]))
        j = int(par[k][j])
        k -= 1
    return np.array(sorted(out), np.int64)


def edge_ranks(keys, N, E):
    ptr = np.zeros(N + 1, np.int64)
    np.cumsum(np.bincount(keys, minlength=N), out=ptr[1:])
    prm = np.argsort(keys, kind="stable")
    r = np.arange(E, dtype=np.int64) - ptr[keys[prm]]
    out = np.empty(E, np.int64)
    out[prm] = r
    return out


def prepare(edge_index, edge_attr, n_nodes):
    row = np.asarray(edge_index[0]).astype(np.int64)
    col = np.asarray(edge_index[1]).astype(np.int64)
    attr16 = np.asarray(edge_attr, dtype=np.float32).astype(np.float16)
    E = row.shape[0]
    N = int(n_nodes)

    rd = np.bincount(row, minlength=N)
    cd = np.bincount(col, minlength=N)
    clD = dp_classes(rd, NCD)
    clE = dp_classes(cd, NCE)
    ncd, nce = len(clD), len(clE)
    dcls = np.searchsorted(clD, np.maximum(rd, 1))
    ecls = np.searchsorted(clE, np.maximum(cd, 1))
    cell = dcls * nce + ecls
    NCELL = ncd * nce

    # Round-robin nodes to cores within each cell: rank r in cell ->
    # core r % 8, slot r // 8. Balances every cell across all cores.
    order = np.lexsort((np.arange(N), cell))
    grp = cell[order]
    starts = np.concatenate([[0], np.nonzero(np.diff(grp))[0] + 1])
    gstart = np.zeros(N, np.int64)
    gstart[starts] = starts
    np.maximum.accumulate(gstart, out=gstart)
    rank = np.arange(N) - gstart
    core = np.empty(N, np.int64)
    kn = np.empty(N, np.int64)
    core[order] = rank % N_CORES
    kn[order] = rank // N_CORES

    g = np.bincount(cell, minlength=NCELL)
    K = -(-(-(-g // N_CORES)) // P)                # ceil(ceil(g/8)/128)
    Dc = clD[np.arange(NCELL) // nce]
    Ec = clE[np.arange(NCELL) % nce]
    cv = np.concatenate([[0], np.cumsum(K)])
    boff = np.concatenate([[0], np.cumsum(K * Ec)])
    WV, WB = int(cv[-1]), int(boff[-1])

    # per D-class totals for the plane-interleaved A stream
    Kd = np.array([K[d * nce:(d + 1) * nce].sum() for d in range(ncd)])
    adoff = np.concatenate([[0], np.cumsum(Kd * clD)])
    WA = int(adoff[-1])

    pn = kn % P
    jn = kn // P

    rrank = edge_ranks(row, N, E)
    crank = edge_ranks(col, N, E)

    # A scatter: plane-interleaved per class. Slot's position within the
    # class = (cv[cell] - cv[class first cell]) + jn.
    svin = cv[cell] - cv[(cell // nce) * nce] + jn
    acol = adoff[dcls[row]] + rrank * Kd[dcls[row]] + svin[row]
    fa = core[row] * (P * WA) + pn[row] * WA + acol
    attr_a = np.zeros((N_CORES, P, WA), np.float16)
    # plane 0 of each class = 1.0 so padded slots get rowsum 1.0 (keeps the
    # scalar-engine reciprocal in range; their outputs are 0 and never read)
    for d in range(ncd):
        attr_a[:, :, adoff[d]:adoff[d] + Kd[d]] = 1.0
    attr_a.reshape(-1)[fa] = attr16

    # B scatter: plane-interleaved per cell.
    bcol = boff[cell[col]] + crank * K[cell[col]] + jn[col]
    fb = core[col] * (P * WB) + pn[col] * WB + bcol
    attr_b = np.zeros(N_CORES * P * WB, np.float16)
    attr_b[fb] = attr16
    attr_b = attr_b.reshape(N_CORES, P, WB)

    classes = []
    for d in range(ncd):
        classes.append(dict(D=int(clD[d]), a0=int(adoff[d]), Kd=int(Kd[d]),
                            v0=int(cv[d * nce]), v1=int(cv[(d + 1) * nce])))
    cells = []
    for c in range(NCELL):
        if K[c] == 0 or Ec[c] == 0:
            continue
        cells.append(dict(E=int(Ec[c]), K=int(K[c]), b0=int(boff[c]),
                          v0=int(cv[c]), d=int(c // nce)))
    geom = dict(WA=WA, WB=WB, WV=WV, classes=classes, cells=cells)
    binfo = (core[col], pn[col] * WB + bcol)
    ident = np.eye(P, dtype=np.float16)
    in_maps = [{"ident": ident, "attr_a": attr_a[c], "attr_b": attr_b[c]}
               for c in range(N_CORES)]
    return in_maps, geom, binfo


def unshard(results, geom, binfo):
    bcore, fb_local = binfo
    outs = np.stack([np.asarray(results[c]["out"]).reshape(-1)
                     for c in range(N_CORES)])
    return outs[bcore, fb_local].astype(np.float32)


# ----------------------------------------------------------------------------
# Device program
# ----------------------------------------------------------------------------

def build_program(geom):
    WA, WB, WV = geom["WA"], geom["WB"], geom["WV"]
    classes = geom["classes"]
    cells = geom["cells"]

    nc = bass.Bass()
    ident_in = nc.declare_dram_parameter("ident", [P, P], F16, isOutput=False)
    attr_a = nc.declare_dram_parameter("attr_a", [P, WA], F16, isOutput=False)
    attr_b = nc.declare_dram_parameter("attr_b", [P, WB], F16, isOutput=False)
    out_ext = nc.declare_dram_parameter("out", [P, WB], F16, isOutput=True)

    achunks = [(w, min(WA, w + CHUNK)) for w in range(0, WA, CHUNK)]
    bchunks = [(w, min(WB, w + CHUNK)) for w in range(0, WB, CHUNK)]
    na = len(achunks)

    def b_hi(ce):
        return (ce["b0"] + ce["E"] * ce["K"] - 1) // CHUNK

    # output store regions at cell boundaries
    regions = []
    tgt = (WB + NREGION - 1) // NREGION
    r0, nmul = 0, 0
    for ci, ce in enumerate(cells):
        end = ce["b0"] + ce["E"] * ce["K"]
        nmul += 1
        if end - r0 >= tgt or ci == len(cells) - 1:
            regions.append((r0, end, nmul))
            r0 = end
    assert regions[-1][1] == WB

    # PSUM segments per class: split [v0, v1) at PSUM bank boundaries
    def segs(cl):
        out = []
        s = cl["v0"]
        while s < cl["v1"]:
            s1 = min(cl["v1"], (s // PSUM_COLS + 1) * PSUM_COLS)
            out.append((s, s1))
            s = s1
        return out

    from contextlib import ExitStack
    with ExitStack() as ctx:
        block = ctx.enter_context(nc.Block())
        sI = ctx.enter_context(nc.semaphore("sI"))
        sA = [ctx.enter_context(nc.semaphore(f"sA{i}"))
              for i in range(len(achunks))]
        sB = [ctx.enter_context(nc.semaphore(f"sB{i}"))
              for i in range(len(bchunks))]
        sRed = ctx.enter_context(nc.semaphore("sRed"))
        sV = ctx.enter_context(nc.semaphore("sV"))
        sMul = ctx.enter_context(nc.semaphore("sMul"))
        sOut = ctx.enter_context(nc.semaphore("sOut"))

        ident = ctx.enter_context(nc.sbuf_tensor("identsb", [P, P], F16))
        A_sb = ctx.enter_context(nc.sbuf_tensor("A_sb", [P, WA], F16))
        B_sb = ctx.enter_context(nc.sbuf_tensor("B_sb", [P, WB], F16))
        ps = ctx.enter_context(nc.psum_tensor("ps", [P, WV], F32))
        psd = ctx.enter_context(nc.psum_tensor("psd", [P, P], F32))
        vh = ctx.enter_context(nc.sbuf_tensor("vh", [P, WV], F16))

        @block.sync
        def _(sync):
            sync.dma_start(out=ident[:, :], in_=ident_in[:, :]).then_inc(sI, 16)
            for i, (w0, w1) in enumerate(achunks):
                sync.dma_start(out=A_sb[:, w0:w1],
                               in_=attr_a[:, w0:w1]).then_inc(sA[i], 16)
            for i, (w0, w1) in enumerate(bchunks):
                sync.dma_start(out=B_sb[:, w0:w1],
                               in_=attr_b[:, w0:w1]).then_inc(sB[i], 16)

        def waiter(eng):
            seen = {}

            def wait(sem, val):
                if seen.get(id(sem), -1) < val:
                    eng.wait_ge(sem, val)
                    seen[id(sem)] = val
            return wait

        @block.tensor
        def _(tensor):
            wait = waiter(tensor)
            wait(sI, 16)  # identity
            for cl in classes:
                D, a0, Kd, v0 = cl["D"], cl["a0"], cl["Kd"], cl["v0"]
                class_segs = segs(cl)
                for si, (s0, s1) in enumerate(class_segs):
                    c0 = a0 + (s0 - v0)
                    w = s1 - s0
                    for i in range(D):
                        # plane i columns for this segment
                        x0 = c0 + i * Kd
                        for ch in range(x0 // CHUNK, (x0 + w - 1) // CHUNK + 1):
                            wait(sA[ch], 16)
                        mm = tensor.matmul(
                            out=ps[:, s0:s1],
                            lhsT=ident[:, :],
                            rhs=A_sb[:, x0:x0 + w],
                            start=(i == 0), stop=(i == D - 1),
                        )
                    if si == len(class_segs) - 1:
                        mm.then_inc(sRed, 1)
            # drain dummy: its completion implies all prior PSUM writes landed
            tensor.matmul(out=psd[:, :P], lhsT=ident[:, :], rhs=ident[:, :],
                          start=True, stop=True).then_inc(sRed, 1)

        @block.vector
        def _(vector):
            wait = waiter(vector)
            for ce in cells:
                for ch in range(ce["b0"] // CHUNK, b_hi(ce) + 1):
                    wait(sB[ch], 16)
                wait(sV, ce["d"] + 1)
                E, K, b0, v0 = ce["E"], ce["K"], ce["b0"], ce["v0"]
                dst = B_sb[:, b0:b0 + E * K].rearrange("p (e k) -> p e k", k=K)
                vector.tensor_tensor(
                    out=dst, in0=vh[:, None, v0:v0 + K].to_broadcast([P, E, K]),
                    in1=dst, op=mybir.AluOpType.mult,
                ).then_inc(sMul, 1)

        @block.scalar
        def _(scalar):
            # all classes + drain dummy: PSUM reads must not race accumulation
            scalar.wait_ge(sRed, len(classes) + 1)
            for d, cl in enumerate(classes):
                v0, v1 = cl["v0"], cl["v1"]
                scalar.add_instruction(mybir.InstActivation(
                    name=nc.get_next_instruction_name(),
                    func=mybir.ActivationFunctionType.Reciprocal,
                    ins=[scalar.lower_ap(ps[:, v0:v1]),
                         mybir.ImmediateValue(dtype=F32, value=0.0),
                         mybir.ImmediateValue(dtype=F32, value=1.0),
                         mybir.ImmediateValue(dtype=F32, value=0.0)],
                    outs=[scalar.lower_ap(vh[:, v0:v1])])).then_inc(sV, 1)
            for r0, r1, nmul in regions:
                scalar.wait_ge(sMul, nmul)
                scalar.dma_start(out=out_ext[:, r0:r1],
                                 in_=B_sb[:, r0:r1]).then_inc(sOut, 16)

    return nc


# ----------------------------------------------------------------------------
# Entry point
# ----------------------------------------------------------------------------

def kernel(edge_index, edge_attr, N):
    import os
    in_maps, geom, binfo = prepare(edge_index, edge_attr, int(N))
    nc = build_program(geom)
    trace = os.environ.get("KTRACE") not in (None, "", "0")
    if trace:
        import types
        import antenv
        if "antenv.axon_hooks" not in sys.modules:
            mod = types.ModuleType("antenv.axon_hooks")
            _h = [None]
            mod.set_axon_ntff_profile_hook = lambda h: _h.__setitem__(0, h)
            mod.get_axon_ntff_profile_hook = lambda: _h[0]
            sys.modules["antenv.axon_hooks"] = mod
            antenv.axon_hooks = mod
            from trn_agent_boot.trn_boot import _ntff_profile_via_ctypes
            mod.set_axon_ntff_profile_hook(
                _ntff_profile_via_ctypes("/opt/axon/libaxon_pjrt.so"))
    res = run_bass_kernel_spmd(nc, in_maps, list(range(N_CORES)), trace=trace)
    kernel.last = (res, in_maps, geom)
    return unshard(res.results, geom, binfo)


if __name__ == "__main__":
    rng = np.random.default_rng(0)
    N = 4096
    E = 65536
    row = np.concatenate([np.arange(N, dtype=np.int32),
                          rng.integers(0, N, E - N, dtype=np.int32)])
    col = rng.integers(0, N, E, dtype=np.int32)
    attr = rng.random(E, dtype=np.float32) * 0.9 + 0.1
    out = kernel(np.stack([row, col]), attr, N)
    rowsum = np.zeros(N, np.float64)
    np.add.at(rowsum, row, attr.astype(np.float64))
    exp = (1.0 / rowsum)[col] * attr
    err = np.abs(out - exp) / np.abs(exp)
    print("max rel err:", err.max())
